# revision 13
# baseline (speedup 1.0000x reference)
"""ALSH Conv2d distributed Trainium2 kernel (8 NeuronCores).

Strategy (v4):
  - Data-parallel over batch: 16 images -> 2 per core, one image per
    64-partition half of SBUF (image0 on partitions 0-63, image1 on 64-127).
  - Conv as 9 shifted K=64 matmuls per output chunk, with the two images'
    matmuls issued as concurrent row-tiled pairs (tile_position (0,0) and
    (64,0)) so the 128x128 PE array stays fully busy. bf16 matmul dtype
    (fp32 accumulation in PSUM).
  - Vote-first: t-matmuls (t[r,m] = sum_c a2[c,r] xp[c,m]) run immediately
    after the input loads, one PSUM bank per chunk with image0 rows on
    partitions 0:9 and image1 on 64:73 (deep 6-buf pool in a nested scope
    released before the conv pool allocates, so neither phase stalls on
    PSUM).  Shifted gather-DMAs + DVE adds + magic-number bucketize give
    the per-core 8-bin histogram by ~25us.
  - The global vote skips the CC stream entirely: 7 XOR-relative
    remote_dma_broadcast descriptors (slot j sends to tpb ^ j) push each
    core's [128,8] partial histogram into peer SBUF rx slots; a monotonic
    remote semaphore counts the 14 lane-arrivals.  XOR is a bijection so
    the slot order is a permutation -- summing the slots plus the local
    partials and a gpsimd partition_all_reduce yields the global histogram
    on every partition with no entry barrier and ~2us of latency.
  - argmax -> one-hot -> factor vector against a host-precomputed 128x8
    factor table (the kernel-side hash depends only on the weights, so it
    is computed bit-exactly on host, jax/cpu).
  - Conv outputs are evicted unscaled to bf16 SBUF staging; as soon as the
    vote lands (~30us, under the conv) 3 row groups x 2 images are scaled
    (DVE/ACT alternating) and stored bf16 over the sync/scalar HWDGE
    queues + gpsimd SWDGE, overlapping the conv tail.  The host converts
    to fp32 and strips the padding during the unshard.
"""

import numpy as np

OC, IC, KS, R_LSH, T_TBL = 128, 64, 3, 2.5, 8
SPAN = KS * KS * IC          # 576
B_FULL, H, W = 16, 56, 56
NCORES = 8
IMG_PER_CORE = B_FULL // NCORES   # 2
HP, WP = H + 2, W + 2        # 58x58 padded grid
GRID = HP * WP               # 3364
MARG = 64                    # zero margin columns on each side of the grid
LCOLS = MARG + GRID + MARG   # 3492
# chunk row counts over the 58 padded rows; all chunks >= 256 cols so fp32r
# runs at full rate (464,464,464,464,464,464,290,290)
CHUNK_ROWS = [8, 8, 8, 8, 8, 8, 5, 5]
ROW_START = [0, 8, 16, 24, 32, 40, 48, 53]
NCHUNK = len(CHUNK_ROWS)
MAXCHUNK = 8 * WP            # 464
SHIFTS = [((r // 3) - 1) * WP + ((r % 3) - 1) for r in range(9)]
# scale/store row groups (in padded rows)
GROUPS = [(0, 15), (15, 30), (30, 44), (44, 58)]
USE_RDMA = True              # False: CC-stream AllGather fallback


def _build_graph(bias_u):
    """Build the 8-core Bass graph. bias_u = (0.5*sum(a[576:]) + b) / R."""
    import concourse.bass as bass
    import concourse.bacc as bacc
    import bass_rust
    import concourse.mybir as mybir
    from concourse import bass_isa
    from concourse import tile

    f32 = mybir.dt.float32
    bf16 = mybir.dt.bfloat16
    AX = mybir.AxisListType.X
    AF = mybir.ActivationFunctionType
    OP = mybir.AluOpType

    nc = bacc.Bacc("TRN2", target_bir_lowering=False, debug=False,
                   num_devices=NCORES, num_swdge_queues=2,
                   monotonic_sem_count=2)

    x_d = nc.dram_tensor("x", [IMG_PER_CORE, IC, LCOLS], bf16, kind="ExternalInput")
    wt_d = nc.dram_tensor("wt", [128, 9 * OC], bf16, kind="ExternalInput")
    a2_d = nc.dram_tensor("a2", [128, 9], bf16, kind="ExternalInput")
    ft_d = nc.dram_tensor("ftT", [OC, T_TBL], f32, kind="ExternalInput")
    io_d = nc.dram_tensor("iota8", [128, T_TBL], f32, kind="ExternalInput")
    out_d = nc.dram_tensor("out", [IMG_PER_CORE, OC, GRID], bf16, kind="ExternalOutput")

    with tile.TileContext(nc) as tc:
        with (
            tc.tile_pool(name="const", bufs=1) as cpool,
            tc.tile_pool(name="xp", bufs=1) as xpool,
            tc.tile_pool(name="tg", bufs=1) as tgpool,
            tc.tile_pool(name="stg", bufs=1) as stgpool,
            tc.tile_pool(name="vv", bufs=9) as vvpool,
            tc.tile_pool(name="vsmall", bufs=1) as vspool,
            tc.tile_pool(name="tiny", bufs=1) as typool,
            tc.tile_pool(name="pswu", bufs=1, space="PSUM") as pswu,
            tc.tile_pool(name="dram", bufs=2, space="DRAM") as dram,
        ):
            # ---- constants ----
            w_sb = cpool.tile([128, 9 * OC], bf16, tag="wsb")
            a2_sb = cpool.tile([128, 9], bf16, tag="a2sb")
            ft_sb = cpool.tile([OC, T_TBL], f32, tag="ftsb")
            iota_sb = cpool.tile([128, T_TBL], f32, tag="iosb")

            # ---- padded input xp: [128, LCOLS]; partitions 0-63 image0,
            #      64-127 image1; grid position m lives at column MARG+m.
            # x arrives host-padded to the full [64, LCOLS] row layout
            # (zero margins + zero-padded 58x58 grid). Split over the three
            # DMA queues so the vote's t-matmul can start ASAP. ----
            xp = xpool.tile([128, LCOLS], bf16, tag="xp")
            HALF = LCOLS // 2
            nc.gpsimd.dma_start(a2_sb[:], a2_d[:])
            nc.sync.dma_start(xp[0:64, 0:HALF], x_d[0][:, 0:HALF])
            nc.scalar.dma_start(xp[0:64, HALF:], x_d[0][:, HALF:])
            nc.gpsimd.dma_start(xp[64:128, 0:HALF], x_d[1][:, 0:HALF])
            nc.sync.dma_start(xp[64:128, HALF:], x_d[1][:, HALF:])
            nc.scalar.dma_start(w_sb[:], wt_d[:])
            nc.sync.dma_start(ft_sb[:], ft_d[:])
            nc.sync.dma_start(iota_sb[:], io_d[:])

            # per-core partial histogram [128, 8]; DVE writes rows 0:112,
            # the pad rows are zeroed once up front
            hist = vspool.tile([128, T_TBL], f32, tag="hist")
            nc.gpsimd.memset(hist[:, :], 0.0)

            # ================= vote stage 1: t[r, m] = sum_c a2[c,r]*xp[c,m] ==
            # one PSUM bank per chunk: image0 rows on partitions 0:9,
            # image1 on 64:73; deep pool in a nested scope (released before
            # the conv pool allocates) so the evict round-trips never stall
            # the PE.
            t_sb = tgpool.tile([128, LCOLS], f32, tag="tsb")
            with tc.tile_pool(name="pst", bufs=6, space="PSUM") as pst:
                for k in range(NCHUNK):
                    ncols = CHUNK_ROWS[k] * WP
                    c0 = MARG + ROW_START[k] * WP
                    tps = pst.tile([128, MAXCHUNK], f32, tag="tps")
                    nc.tensor.matmul(tps[0:9, 0:ncols],
                                     a2_sb[0:64, :],
                                     xp[0:64, c0:c0 + ncols],
                                     start=True, stop=True, tile_position=(0, 0))
                    nc.tensor.matmul(tps[64:73, 0:ncols],
                                     a2_sb[64:128, :],
                                     xp[64:128, c0:c0 + ncols],
                                     start=True, stop=True, tile_position=(64, 64))
                    nc.vector.tensor_copy(t_sb[0:9, c0:c0 + ncols], tps[0:9, 0:ncols])
                    nc.scalar.copy(t_sb[64:73, c0:c0 + ncols], tps[64:73, 0:ncols])
            del tps

            # ---- shifted gathers: vvr[y, x] = t[r, (y+1+dy)*58 + (x+1+dx)] ----
            # split across the sync/scalar HWDGE queues + gpsimd SWDGE
            vvr_tiles = []
            for r in range(9):
                vvr = vvpool.tile([112, W], f32, tag="vvr")
                off = MARG + SHIFTS[r] + WP + 1
                for i in range(IMG_PER_CORE):
                    src = t_sb[64 * i + r:64 * i + r + 1, off:off + H * WP] \
                        .rearrange("p (y x) -> p y x", x=WP)[:, :, 0:W]
                    eng = (nc.sync, nc.gpsimd, nc.scalar)[(2 * r + i) % 3]
                    eng.dma_start(vvr[56 * i:56 * i + 56, :], src)
                vvr_tiles.append(vvr)

            # ---- v = sum_r vvr ; bucketize ; histogram (DVE/ACT, pre-conv) --
            # floor via the magic-number round-to-nearest trick (no floor/mod
            # ALU op): rni(z) = (z + MAGIC) - MAGIC for |z| < 2^22.
            MAGIC = 12582912.0  # 1.5 * 2^23
            acc = vspool.tile([112, W], f32, tag="acc")
            nc.vector.tensor_tensor(acc[:], vvr_tiles[0][:], vvr_tiles[1][:], OP.add)
            for r in range(2, 9):
                nc.vector.tensor_tensor(acc[:], acc[:], vvr_tiles[r][:], OP.add)
            u_t = vspool.tile([112, W], f32, tag="ut")
            nc.vector.tensor_scalar(u_t[:], acc[:], float(1.0 / R_LSH), float(bias_u), OP.mult, OP.add)
            u2 = vspool.tile([112, W], f32, tag="u2")
            nc.vector.tensor_scalar(u2[:], u_t[:], 0.49995, MAGIC, OP.subtract, OP.add)
            q_t = vspool.tile([112, W], f32, tag="qt")
            nc.vector.tensor_scalar(q_t[:], u2[:], MAGIC, None, OP.subtract)
            aq = vspool.tile([112, W], f32, tag="aq")
            nc.vector.scalar_tensor_tensor(aq[:], q_t[:], -1.0, q_t[:], OP.mult, OP.max)
            d1 = vspool.tile([112, W], f32, tag="d1")
            nc.vector.tensor_scalar(d1[:], aq[:], 0.125, 0.499, OP.mult, OP.subtract)
            d2 = vspool.tile([112, W], f32, tag="d2")
            nc.vector.tensor_scalar(d2[:], d1[:], MAGIC, MAGIC, OP.add, OP.subtract)
            votes = vspool.tile([112, W], f32, tag="votes")
            nc.vector.scalar_tensor_tensor(votes[:], d2[:], -8.0, aq[:], OP.mult, OP.add)

            eq = vspool.tile([112, W], f32, tag="eq")
            for t in range(T_TBL):
                nc.vector.tensor_scalar(eq[:], votes[:], float(t), None, OP.is_equal)
                nc.vector.reduce_sum(hist[0:112, t:t + 1], eq[:], AX)

            tot = typool.tile([128, T_TBL], f32, tag="tot")
            if USE_RDMA:
                # ---- global histogram via XOR-relative remote DMA ----
                # instruction j broadcasts this core's [128,8] partials to
                # tpb^j's rx slot j-1; 7 instructions x 2 lanes bump the
                # monotonic remote sem to 14 once every peer's data landed.
                rx = vspool.tile([128, (NCORES - 1) * T_TBL], f32, tag="rx")
                mono_rx = nc.monotonic_semaphore(0)
                mono_loc = nc.monotonic_semaphore(1)
                for j in range(1, NCORES):
                    rdests = [(0, k) if k == j else None for k in range(NCORES)]
                    nc.gpsimd.remote_dma_broadcast(
                        rx[:, (j - 1) * T_TBL:j * T_TBL],
                        hist[:, :],
                        mono_rx.sem(),
                        mono_loc.sem(),
                        rdests=rdests,
                        queue_num=1,
                    )
                nc.gpsimd.trigger_dma(count=None, queue_num=1)
            else:
                hist_row = vspool.tile([1, T_TBL], f32, tag="histrow")
                nc.gpsimd.reduce_sum(hist_row[0:1, :], hist[0:112, :], mybir.AxisListType.C)
                cin = dram.tile([1, T_TBL], f32, tag="cin")
                cout = dram.tile([NCORES, T_TBL], f32, tag="cout", addr_space="Shared")
                nc.gpsimd.dma_start(cin[:], hist_row[0:1, :])
                nc.gpsimd.collective_compute(
                    "AllGather",
                    mybir.AluOpType.bypass,
                    replica_groups=[list(range(NCORES))],
                    ins=[cin[:].opt()],
                    outs=[cout[:].opt()],
                )
                hs_bc = typool.tile([128, NCORES * T_TBL], f32, tag="hsbc")
                nc.sync.dma_start(hs_bc[:], cout[:].rearrange("r t -> (r t)")
                                  .unsqueeze(0).broadcast_to([128, NCORES * T_TBL]))

            # ---- PE warm-up between the t-matmuls and the conv so the HAM
            # clock gate is fully open before the conv begins ----
            wups = pswu.tile([128, MAXCHUNK], f32, tag="pswu")
            for wi in range(8):
                nc.tensor.matmul(wups[:, 0:MAXCHUNK],
                                 w_sb[0:128, 0:128],
                                 w_sb[0:128, 0:MAXCHUNK],
                                 start=True, stop=True)

            # ================= main conv =================
            # per-image contiguous bf16 staging over the whole padded grid
            stg_img0 = stgpool.tile([128, GRID], bf16, tag="stg0")
            stg_img1 = stgpool.tile([128, GRID], bf16, tag="stg1")
            stg_imgs = [stg_img0, stg_img1]
            with tc.tile_pool(name="psc", bufs=6, space="PSUM") as psc:
                for k in range(NCHUNK):
                    ncols = CHUNK_ROWS[k] * WP
                    c0 = MARG + ROW_START[k] * WP
                    g0 = ROW_START[k] * WP
                    psA = psc.tile([128, MAXCHUNK], f32, tag="psconv")
                    psB = psc.tile([128, MAXCHUNK], f32, tag="psconv")
                    for r in range(9):
                        s = SHIFTS[r]
                        nc.tensor.matmul(psA[:, 0:ncols],
                                         w_sb[0:64, r * OC:(r + 1) * OC],
                                         xp[0:64, c0 + s:c0 + s + ncols],
                                         start=(r == 0), stop=(r == 8),
                                         tile_position=(0, 0))
                        nc.tensor.matmul(psB[:, 0:ncols],
                                         w_sb[64:128, r * OC:(r + 1) * OC],
                                         xp[64:128, c0 + s:c0 + s + ncols],
                                         start=(r == 0), stop=(r == 8),
                                         tile_position=(64, 0))
                    # both evictions on ACT: the DVE stays clear for the
                    # vote/histogram chain feeding the remote exchange
                    nc.scalar.copy(stg_imgs[0][:, g0:g0 + ncols], psA[:, 0:ncols])
                    nc.scalar.copy(stg_imgs[1][:, g0:g0 + ncols], psB[:, 0:ncols])

            # ---- complete the vote ----
            if USE_RDMA:
                # wait for all 7 peers' data, then sum the permuted slots
                # plus the local partials; the no-sync barrier pins the
                # gpsimd consumers behind the manual semaphore wait (Tile
                # cannot see the remote writes).
                mono_rx.wait_inc(14)
                tc.no_sync_barrier()
                nc.gpsimd.tensor_tensor(tot[:], rx[:, 0:T_TBL],
                                        rx[:, T_TBL:2 * T_TBL], OP.add)
                for j in range(3, NCORES):
                    nc.gpsimd.tensor_tensor(tot[:], tot[:],
                                            rx[:, (j - 1) * T_TBL:j * T_TBL], OP.add)
                nc.gpsimd.tensor_tensor(tot[:], tot[:], hist[:], OP.add)
                tot_all = typool.tile([128, T_TBL], f32, tag="totall")
                nc.gpsimd.partition_all_reduce(tot_all[:], tot[:], 128,
                                               bass_isa.ReduceOp.add)
            else:
                hs_v = hs_bc[:, :].rearrange("p (r t) -> p t r", t=T_TBL)
                tot_all = typool.tile([128, T_TBL], f32, tag="totall")
                nc.vector.reduce_sum(tot_all[:], hs_v, AX)

            # ---- argmax -> one-hot -> factor vector ----
            score = typool.tile([128, T_TBL], f32, tag="score")
            nc.vector.scalar_tensor_tensor(score[:], tot_all[:], float(T_TBL), iota_sb[:],
                                           OP.mult, OP.subtract)
            mx = typool.tile([128, 1], f32, tag="mx")
            nc.vector.reduce_max(mx[:], score[:], AX)
            eqb = typool.tile([128, T_TBL], f32, tag="eqb")
            nc.vector.tensor_scalar(eqb[:], score[:], mx[:, 0:1], None, OP.is_equal)
            fvt = typool.tile([128, T_TBL], f32, tag="fvt")
            nc.vector.tensor_tensor(fvt[:], ft_sb[:], eqb[:], OP.mult)
            fv_sb = typool.tile([128, 1], f32, tag="fvsb")
            nc.vector.reduce_sum(fv_sb[:], fvt[:], AX)

            # ---- scale by factor vector, then DMA out; 3 row groups x 2
            # images pipelined over DVE/ACT and three DMA queues ----
            ei = 0
            for gi, (r0, r1) in enumerate(GROUPS):
                for i in range(IMG_PER_CORE):
                    stg = stg_imgs[i]
                    if ei % 2 == 0:
                        nc.vector.tensor_scalar(stg[:, r0 * WP:r1 * WP],
                                                stg[:, r0 * WP:r1 * WP],
                                                fv_sb[:, 0:1], None, OP.mult)
                    else:
                        nc.scalar.activation(stg[:, r0 * WP:r1 * WP],
                                             stg[:, r0 * WP:r1 * WP],
                                             AF.Copy, scale=fv_sb[:, 0:1])
                    # contiguous padded-grid store; host strips the padding
                    oeng = (nc.sync, nc.scalar)[ei % 2]
                    oeng.dma_start(out_d[i, :, r0 * WP:r1 * WP],
                                   stg[:, r0 * WP:r1 * WP])
                    ei += 1

    nc.compile()
    return nc


def _host_prep(kernels, a, b):
    """Host-side weight layouts + bit-exact factor table via jax on CPU."""
    import jax
    import jax.numpy as jnp

    cpu = jax.devices("cpu")[0]
    k_j = jax.device_put(jnp.asarray(kernels, jnp.float32), cpu)
    a_j = jax.device_put(jnp.asarray(a, jnp.float32), cpu)
    b_j = jax.device_put(jnp.asarray(b, jnp.float32), cpu)

    norms2 = jnp.sum(k_j * k_j, axis=1)
    powers = jnp.stack([norms2 ** (2 ** i) for i in range(5)], axis=1)
    hk = k_j @ a_j[:SPAN] + powers @ a_j[SPAN:]
    kidx = np.asarray(jnp.abs(jnp.fmod(jnp.floor((hk + b_j) / R_LSH).astype(jnp.int32), T_TBL)))

    ftT = np.zeros((T_TBL, OC), np.float32)
    for t in range(T_TBL):
        mask = (kidx == t).astype(np.float32)
        cnt = mask.sum()
        if cnt > 0:
            ftT[t] = mask * np.float32(OC / max(cnt, np.float32(1.0)))
        else:
            ftT[t] = 1.0
    ftT = np.ascontiguousarray(ftT.T)  # [OC, T_TBL], oc on partitions

    c0 = 0.5 * float(jnp.sum(a_j[SPAN:]))
    bias_u = (c0 + float(b_j)) / R_LSH

    import ml_dtypes
    wt_half = np.asarray(kernels, np.float32).reshape(OC, IC, 9).transpose(1, 2, 0)  # [64, 9, 128]
    wt = np.concatenate([wt_half, wt_half], axis=0).reshape(128, 9 * OC)
    wt = np.ascontiguousarray(wt.astype(ml_dtypes.bfloat16))

    a2_half = np.asarray(a, np.float32)[:SPAN].reshape(IC, 9)
    a2 = np.ascontiguousarray(
        np.concatenate([a2_half, a2_half], axis=0).astype(ml_dtypes.bfloat16))

    iota8 = np.ascontiguousarray(np.tile(np.arange(T_TBL, dtype=np.float32), (128, 1)))
    return wt, a2, ftT, iota8, bias_u


def _pad_shard(xs):
    """[n, 64, 56, 56] -> bf16 [n, 64, LCOLS]: margins + padded 58x58 grid."""
    import ml_dtypes
    n = xs.shape[0]
    out = np.zeros((n, IC, LCOLS), ml_dtypes.bfloat16)
    grid = np.pad(xs, ((0, 0), (0, 0), (1, 1), (1, 1)))
    out[:, :, MARG:MARG + GRID] = grid.reshape(n, IC, GRID).astype(ml_dtypes.bfloat16)
    return np.ascontiguousarray(out)


def _in_maps(x, kernels, a, b):
    wt, a2, ftT, iota8, bias_u = _host_prep(kernels, a, b)
    in_maps = []
    for c in range(NCORES):
        in_maps.append({
            "x": _pad_shard(x[IMG_PER_CORE * c:IMG_PER_CORE * (c + 1)]),
            "wt": wt,
            "a2": a2,
            "ftT": ftT,
            "iota8": iota8,
        })
    return in_maps, bias_u


def _unshard(res):
    """Gather per-core padded bf16 outputs -> full fp32 [16, OC, 56, 56]."""
    out_pad = np.concatenate(
        [np.asarray(res.results[c]["out"], dtype=np.float32) for c in range(NCORES)],
        axis=0)
    return np.ascontiguousarray(
        out_pad.reshape(B_FULL, OC, HP, WP)[:, :, 1:1 + H, 1:1 + W])


def kernel(x, kernels, a, b, mode=0, **_ignored):
    from concourse.bass_utils import run_bass_kernel_spmd

    x = np.ascontiguousarray(np.asarray(x, np.float32))
    kernels = np.asarray(kernels, np.float32)
    a = np.asarray(a, np.float32)

    in_maps, bias_u = _in_maps(x, kernels, a, b)
    nc = _build_graph(bias_u)
    res = run_bass_kernel_spmd(nc, in_maps, core_ids=list(range(NCORES)))
    return _unshard(res)


# revision 15
# speedup vs baseline: 1.0059x; 1.0059x over previous
"""ALSH Conv2d distributed Trainium2 kernel (8 NeuronCores).

Strategy (v4):
  - Data-parallel over batch: 16 images -> 2 per core, one image per
    64-partition half of SBUF (image0 on partitions 0-63, image1 on 64-127).
  - Conv as 9 shifted K=64 matmuls per output chunk, with the two images'
    matmuls issued as concurrent row-tiled pairs (tile_position (0,0) and
    (64,0)) so the 128x128 PE array stays fully busy. bf16 matmul dtype
    (fp32 accumulation in PSUM).
  - Vote-first: t-matmuls (t[r,m] = sum_c a2[c,r] xp[c,m]) run immediately
    after the input loads, one PSUM bank per chunk with image0 rows on
    partitions 0:9 and image1 on 64:73 (deep 6-buf pool in a nested scope
    released before the conv pool allocates, so neither phase stalls on
    PSUM).  Shifted gather-DMAs + DVE adds + magic-number bucketize give
    the per-core 8-bin histogram by ~25us.
  - The global vote skips the CC stream entirely: 7 XOR-relative
    remote_dma_broadcast descriptors (slot j sends to tpb ^ j) push each
    core's [128,8] partial histogram into peer SBUF rx slots; a monotonic
    remote semaphore counts the 14 lane-arrivals.  XOR is a bijection so
    the slot order is a permutation -- summing the slots plus the local
    partials and a gpsimd partition_all_reduce yields the global histogram
    on every partition with no entry barrier and ~2us of latency.
  - argmax -> one-hot -> factor vector against a host-precomputed 128x8
    factor table (the kernel-side hash depends only on the weights, so it
    is computed bit-exactly on host, jax/cpu).
  - Conv outputs are evicted unscaled to bf16 SBUF staging; as soon as the
    vote lands (~30us, under the conv) 3 row groups x 2 images are scaled
    (DVE/ACT alternating) and stored bf16 over the sync/scalar HWDGE
    queues + gpsimd SWDGE, overlapping the conv tail.  The host converts
    to fp32 and strips the padding during the unshard.
"""

import numpy as np

OC, IC, KS, R_LSH, T_TBL = 128, 64, 3, 2.5, 8
SPAN = KS * KS * IC          # 576
B_FULL, H, W = 16, 56, 56
NCORES = 8
IMG_PER_CORE = B_FULL // NCORES   # 2
HP, WP = H + 2, W + 2        # 58x58 padded grid
GRID = HP * WP               # 3364
MARG = 64                    # zero margin columns on each side of the grid
LCOLS = MARG + GRID + MARG   # 3492
# chunk row counts over the 58 padded rows; all chunks >= 256 cols so fp32r
# runs at full rate (464,464,464,464,464,464,290,290)
CHUNK_ROWS = [8, 8, 8, 8, 8, 8, 5, 5]
ROW_START = [0, 8, 16, 24, 32, 40, 48, 53]
NCHUNK = len(CHUNK_ROWS)
MAXCHUNK = 8 * WP            # 464
SHIFTS = [((r // 3) - 1) * WP + ((r % 3) - 1) for r in range(9)]
# scale/store row groups (in padded rows)
GROUPS = [(0, 15), (15, 30), (30, 44), (44, 58)]
USE_RDMA = True              # False: CC-stream AllGather fallback


def _build_graph(bias_u):
    """Build the 8-core Bass graph. bias_u = (0.5*sum(a[576:]) + b) / R."""
    import concourse.bass as bass
    import concourse.bacc as bacc
    import bass_rust
    import concourse.mybir as mybir
    from concourse import bass_isa
    from concourse import tile

    f32 = mybir.dt.float32
    bf16 = mybir.dt.bfloat16
    AX = mybir.AxisListType.X
    AF = mybir.ActivationFunctionType
    OP = mybir.AluOpType

    nc = bacc.Bacc("TRN2", target_bir_lowering=False, debug=False,
                   num_devices=NCORES, num_swdge_queues=2,
                   monotonic_sem_count=2)

    x_d = nc.dram_tensor("x", [IMG_PER_CORE, IC, LCOLS], bf16, kind="ExternalInput")
    wt_d = nc.dram_tensor("wt", [128, 9 * OC], bf16, kind="ExternalInput")
    a2_d = nc.dram_tensor("a2", [128, 9], bf16, kind="ExternalInput")
    ft_d = nc.dram_tensor("ftT", [OC, T_TBL], f32, kind="ExternalInput")
    io_d = nc.dram_tensor("iota8", [128, T_TBL], f32, kind="ExternalInput")
    out_d = nc.dram_tensor("out", [IMG_PER_CORE, OC, GRID], bf16, kind="ExternalOutput")

    with tile.TileContext(nc) as tc:
        with (
            tc.tile_pool(name="const", bufs=1) as cpool,
            tc.tile_pool(name="xp", bufs=1) as xpool,
            tc.tile_pool(name="tg", bufs=1) as tgpool,
            tc.tile_pool(name="stg", bufs=1) as stgpool,
            tc.tile_pool(name="vv", bufs=9) as vvpool,
            tc.tile_pool(name="vsmall", bufs=1) as vspool,
            tc.tile_pool(name="tiny", bufs=1) as typool,
            tc.tile_pool(name="pswu", bufs=1, space="PSUM") as pswu,
            tc.tile_pool(name="dram", bufs=2, space="DRAM") as dram,
        ):
            # ---- constants ----
            w_sb = cpool.tile([128, 9 * OC], bf16, tag="wsb")
            a2_sb = cpool.tile([128, 9], bf16, tag="a2sb")
            ft_sb = cpool.tile([OC, T_TBL], f32, tag="ftsb")
            iota_sb = cpool.tile([128, T_TBL], f32, tag="iosb")

            # ---- padded input xp: [128, LCOLS]; partitions 0-63 image0,
            #      64-127 image1; grid position m lives at column MARG+m.
            # x arrives host-padded to the full [64, LCOLS] row layout
            # (zero margins + zero-padded 58x58 grid). Split over the three
            # DMA queues so the vote's t-matmul can start ASAP. ----
            xp = xpool.tile([128, LCOLS], bf16, tag="xp")
            HALF = LCOLS // 2
            nc.gpsimd.dma_start(a2_sb[:], a2_d[:])
            nc.sync.dma_start(xp[0:64, 0:HALF], x_d[0][:, 0:HALF])
            nc.scalar.dma_start(xp[0:64, HALF:], x_d[0][:, HALF:])
            nc.gpsimd.dma_start(xp[64:128, 0:HALF], x_d[1][:, 0:HALF])
            nc.sync.dma_start(xp[64:128, HALF:], x_d[1][:, HALF:])
            nc.scalar.dma_start(w_sb[:], wt_d[:])
            nc.sync.dma_start(ft_sb[:], ft_d[:])
            nc.sync.dma_start(iota_sb[:], io_d[:])

            # per-core partial histogram [128, 8]; DVE writes rows 0:112,
            # the pad rows are zeroed once up front
            hist = vspool.tile([128, T_TBL], f32, tag="hist")
            nc.gpsimd.memset(hist[:, :], 0.0)

            # ================= vote stage 1: t[r, m] = sum_c a2[c,r]*xp[c,m] ==
            # one PSUM bank per chunk: image0 rows on partitions 0:9,
            # image1 on 64:73; deep pool in a nested scope (released before
            # the conv pool allocates) so the evict round-trips never stall
            # the PE.
            t_sb = tgpool.tile([128, LCOLS], f32, tag="tsb")
            with tc.tile_pool(name="pst", bufs=6, space="PSUM") as pst:
                for k in range(NCHUNK):
                    ncols = CHUNK_ROWS[k] * WP
                    c0 = MARG + ROW_START[k] * WP
                    tps = pst.tile([128, MAXCHUNK], f32, tag="tps")
                    nc.tensor.matmul(tps[0:9, 0:ncols],
                                     a2_sb[0:64, :],
                                     xp[0:64, c0:c0 + ncols],
                                     start=True, stop=True, tile_position=(0, 0))
                    nc.tensor.matmul(tps[64:73, 0:ncols],
                                     a2_sb[64:128, :],
                                     xp[64:128, c0:c0 + ncols],
                                     start=True, stop=True, tile_position=(64, 64))
                    nc.vector.tensor_copy(t_sb[0:9, c0:c0 + ncols], tps[0:9, 0:ncols])
                    nc.scalar.copy(t_sb[64:73, c0:c0 + ncols], tps[64:73, 0:ncols])
            del tps

            # ---- shifted gathers: vvr[y, x] = t[r, (y+1+dy)*58 + (x+1+dx)] ----
            # split across the sync/scalar HWDGE queues + gpsimd SWDGE
            vvr_tiles = []
            for r in range(9):
                vvr = vvpool.tile([112, W], f32, tag="vvr")
                off = MARG + SHIFTS[r] + WP + 1
                for i in range(IMG_PER_CORE):
                    src = t_sb[64 * i + r:64 * i + r + 1, off:off + H * WP] \
                        .rearrange("p (y x) -> p y x", x=WP)[:, :, 0:W]
                    eng = (nc.sync, nc.gpsimd, nc.scalar)[(2 * r + i) % 3]
                    eng.dma_start(vvr[56 * i:56 * i + 56, :], src)
                vvr_tiles.append(vvr)

            # ---- v = sum_r vvr ; bucketize ; histogram (DVE/ACT, pre-conv) --
            # floor via the magic-number round-to-nearest trick (no floor/mod
            # ALU op): rni(z) = (z + MAGIC) - MAGIC for |z| < 2^22.
            MAGIC = 12582912.0  # 1.5 * 2^23
            acc = vspool.tile([112, W], f32, tag="acc")
            nc.vector.tensor_tensor(acc[:], vvr_tiles[0][:], vvr_tiles[1][:], OP.add)
            for r in range(2, 9):
                nc.vector.tensor_tensor(acc[:], acc[:], vvr_tiles[r][:], OP.add)
            u_t = vspool.tile([112, W], f32, tag="ut")
            nc.vector.tensor_scalar(u_t[:], acc[:], float(1.0 / R_LSH), float(bias_u), OP.mult, OP.add)
            u2 = vspool.tile([112, W], f32, tag="u2")
            nc.vector.tensor_scalar(u2[:], u_t[:], 0.49995, MAGIC, OP.subtract, OP.add)
            q_t = vspool.tile([112, W], f32, tag="qt")
            nc.vector.tensor_scalar(q_t[:], u2[:], MAGIC, None, OP.subtract)
            aq = vspool.tile([112, W], f32, tag="aq")
            nc.vector.scalar_tensor_tensor(aq[:], q_t[:], -1.0, q_t[:], OP.mult, OP.max)
            d1 = vspool.tile([112, W], f32, tag="d1")
            nc.vector.tensor_scalar(d1[:], aq[:], 0.125, 0.499, OP.mult, OP.subtract)
            d2 = vspool.tile([112, W], f32, tag="d2")
            nc.vector.tensor_scalar(d2[:], d1[:], MAGIC, MAGIC, OP.add, OP.subtract)
            votes = vspool.tile([112, W], f32, tag="votes")
            nc.vector.scalar_tensor_tensor(votes[:], d2[:], -8.0, aq[:], OP.mult, OP.add)

            eq = vspool.tile([112, W], f32, tag="eq")
            for t in range(T_TBL):
                nc.vector.tensor_scalar(eq[:], votes[:], float(t), None, OP.is_equal)
                nc.vector.reduce_sum(hist[0:112, t:t + 1], eq[:], AX)

            tot = typool.tile([128, T_TBL], f32, tag="tot")
            if USE_RDMA:
                # ---- global histogram via XOR-relative remote DMA ----
                # instruction j broadcasts this core's [128,8] partials to
                # tpb^j's rx slot j-1; 7 instructions x 2 lanes bump the
                # monotonic remote sem to 14 once every peer's data landed.
                rx = vspool.tile([128, (NCORES - 1) * T_TBL], f32, tag="rx")
                mono_rx = nc.monotonic_semaphore(0)
                mono_loc = nc.monotonic_semaphore(1)
                for j in range(1, NCORES):
                    rdests = [(0, k) if k == j else None for k in range(NCORES)]
                    nc.gpsimd.remote_dma_broadcast(
                        rx[:, (j - 1) * T_TBL:j * T_TBL],
                        hist[:, :],
                        mono_rx.sem(),
                        mono_loc.sem(),
                        rdests=rdests,
                        queue_num=1,
                    )
                nc.gpsimd.trigger_dma(count=None, queue_num=1)
            else:
                hist_row = vspool.tile([1, T_TBL], f32, tag="histrow")
                nc.gpsimd.reduce_sum(hist_row[0:1, :], hist[0:112, :], mybir.AxisListType.C)
                cin = dram.tile([1, T_TBL], f32, tag="cin")
                cout = dram.tile([NCORES, T_TBL], f32, tag="cout", addr_space="Shared")
                nc.gpsimd.dma_start(cin[:], hist_row[0:1, :])
                nc.gpsimd.collective_compute(
                    "AllGather",
                    mybir.AluOpType.bypass,
                    replica_groups=[list(range(NCORES))],
                    ins=[cin[:].opt()],
                    outs=[cout[:].opt()],
                )
                hs_bc = typool.tile([128, NCORES * T_TBL], f32, tag="hsbc")
                nc.sync.dma_start(hs_bc[:], cout[:].rearrange("r t -> (r t)")
                                  .unsqueeze(0).broadcast_to([128, NCORES * T_TBL]))

            # ---- PE warm-up between the t-matmuls and the conv so the HAM
            # clock gate is fully open before the conv begins ----
            wups = pswu.tile([128, MAXCHUNK], f32, tag="pswu")
            for wi in range(8):
                nc.tensor.matmul(wups[:, 0:MAXCHUNK],
                                 w_sb[0:128, 0:128],
                                 w_sb[0:128, 0:MAXCHUNK],
                                 start=True, stop=True)

            # ================= main conv =================
            # per-image contiguous bf16 staging over the whole padded grid
            stg_img0 = stgpool.tile([128, GRID], bf16, tag="stg0")
            stg_img1 = stgpool.tile([128, GRID], bf16, tag="stg1")
            stg_imgs = [stg_img0, stg_img1]
            with tc.tile_pool(name="psc", bufs=6, space="PSUM") as psc:
                for k in range(NCHUNK):
                    ncols = CHUNK_ROWS[k] * WP
                    c0 = MARG + ROW_START[k] * WP
                    g0 = ROW_START[k] * WP
                    psA = psc.tile([128, MAXCHUNK], f32, tag="psconv")
                    psB = psc.tile([128, MAXCHUNK], f32, tag="psconv")
                    for r in range(9):
                        s = SHIFTS[r]
                        nc.tensor.matmul(psA[:, 0:ncols],
                                         w_sb[0:64, r * OC:(r + 1) * OC],
                                         xp[0:64, c0 + s:c0 + s + ncols],
                                         start=(r == 0), stop=(r == 8),
                                         tile_position=(0, 0))
                        nc.tensor.matmul(psB[:, 0:ncols],
                                         w_sb[64:128, r * OC:(r + 1) * OC],
                                         xp[64:128, c0 + s:c0 + s + ncols],
                                         start=(r == 0), stop=(r == 8),
                                         tile_position=(64, 0))
                    # both evictions on ACT: the DVE stays clear for the
                    # vote/histogram chain feeding the remote exchange
                    nc.scalar.copy(stg_imgs[0][:, g0:g0 + ncols], psA[:, 0:ncols])
                    nc.scalar.copy(stg_imgs[1][:, g0:g0 + ncols], psB[:, 0:ncols])

            # ---- complete the vote ----
            if USE_RDMA:
                # wait for all 7 peers' data, then sum the permuted slots
                # plus the local partials; the no-sync barrier pins the
                # gpsimd consumers behind the manual semaphore wait (Tile
                # cannot see the remote writes).
                mono_rx.wait_inc(14)
                tc.no_sync_barrier()
                nc.gpsimd.tensor_tensor(tot[:], rx[:, 0:T_TBL],
                                        rx[:, T_TBL:2 * T_TBL], OP.add)
                for j in range(3, NCORES):
                    nc.gpsimd.tensor_tensor(tot[:], tot[:],
                                            rx[:, (j - 1) * T_TBL:j * T_TBL], OP.add)
                nc.gpsimd.tensor_tensor(tot[:], tot[:], hist[:], OP.add)
                tot_all = typool.tile([128, T_TBL], f32, tag="totall")
                nc.gpsimd.partition_all_reduce(tot_all[:], tot[:], 128,
                                               bass_isa.ReduceOp.add)
            else:
                hs_v = hs_bc[:, :].rearrange("p (r t) -> p t r", t=T_TBL)
                tot_all = typool.tile([128, T_TBL], f32, tag="totall")
                nc.vector.reduce_sum(tot_all[:], hs_v, AX)

            # ---- argmax -> one-hot -> factor vector ----
            score = typool.tile([128, T_TBL], f32, tag="score")
            nc.vector.scalar_tensor_tensor(score[:], tot_all[:], float(T_TBL), iota_sb[:],
                                           OP.mult, OP.subtract)
            mx = typool.tile([128, 1], f32, tag="mx")
            nc.vector.reduce_max(mx[:], score[:], AX)
            eqb = typool.tile([128, T_TBL], f32, tag="eqb")
            nc.vector.tensor_scalar(eqb[:], score[:], mx[:, 0:1], None, OP.is_equal)
            fvt = typool.tile([128, T_TBL], f32, tag="fvt")
            nc.vector.tensor_tensor(fvt[:], ft_sb[:], eqb[:], OP.mult)
            fv_sb = typool.tile([128, 1], f32, tag="fvsb")
            nc.vector.reduce_sum(fv_sb[:], fvt[:], AX)

            # ---- scale by factor vector, then DMA out; 3 row groups x 2
            # images pipelined over DVE/ACT and three DMA queues ----
            ei = 0
            for gi, (r0, r1) in enumerate(GROUPS):
                for i in range(IMG_PER_CORE):
                    stg = stg_imgs[i]
                    if ei % 2 == 0:
                        nc.vector.tensor_scalar(stg[:, r0 * WP:r1 * WP],
                                                stg[:, r0 * WP:r1 * WP],
                                                fv_sb[:, 0:1], None, OP.mult)
                    else:
                        nc.scalar.activation(stg[:, r0 * WP:r1 * WP],
                                             stg[:, r0 * WP:r1 * WP],
                                             AF.Copy, scale=fv_sb[:, 0:1])
                    # contiguous padded-grid store; host strips the padding
                    oeng = (nc.sync, nc.scalar)[ei % 2]
                    oeng.dma_start(out_d[i, :, r0 * WP:r1 * WP],
                                   stg[:, r0 * WP:r1 * WP])
                    ei += 1

    nc.compile()
    return nc


def _host_prep(kernels, a, b):
    """Host-side weight layouts + bit-exact factor table via jax on CPU."""
    import jax
    import jax.numpy as jnp

    cpu = jax.devices("cpu")[0]
    k_j = jax.device_put(jnp.asarray(kernels, jnp.float32), cpu)
    a_j = jax.device_put(jnp.asarray(a, jnp.float32), cpu)
    b_j = jax.device_put(jnp.asarray(b, jnp.float32), cpu)

    norms2 = jnp.sum(k_j * k_j, axis=1)
    powers = jnp.stack([norms2 ** (2 ** i) for i in range(5)], axis=1)
    hk = k_j @ a_j[:SPAN] + powers @ a_j[SPAN:]
    kidx = np.asarray(jnp.abs(jnp.fmod(jnp.floor((hk + b_j) / R_LSH).astype(jnp.int32), T_TBL)))

    ftT = np.zeros((T_TBL, OC), np.float32)
    for t in range(T_TBL):
        mask = (kidx == t).astype(np.float32)
        cnt = mask.sum()
        if cnt > 0:
            ftT[t] = mask * np.float32(OC / max(cnt, np.float32(1.0)))
        else:
            ftT[t] = 1.0
    ftT = np.ascontiguousarray(ftT.T)  # [OC, T_TBL], oc on partitions

    c0 = 0.5 * float(jnp.sum(a_j[SPAN:]))
    bias_u = (c0 + float(b_j)) / R_LSH

    import ml_dtypes
    wt_half = np.asarray(kernels, np.float32).reshape(OC, IC, 9).transpose(1, 2, 0)  # [64, 9, 128]
    wt = np.concatenate([wt_half, wt_half], axis=0).reshape(128, 9 * OC)
    wt = np.ascontiguousarray(wt.astype(ml_dtypes.bfloat16))

    a2_half = np.asarray(a, np.float32)[:SPAN].reshape(IC, 9)
    a2 = np.ascontiguousarray(
        np.concatenate([a2_half, a2_half], axis=0).astype(ml_dtypes.bfloat16))

    iota8 = np.ascontiguousarray(np.tile(np.arange(T_TBL, dtype=np.float32), (128, 1)))
    return wt, a2, ftT, iota8, bias_u


def _pad_shard(xs):
    """[n, 64, 56, 56] -> bf16 [n, 64, LCOLS]: margins + padded 58x58 grid."""
    import ml_dtypes
    n = xs.shape[0]
    out = np.zeros((n, IC, LCOLS), ml_dtypes.bfloat16)
    grid = np.pad(xs, ((0, 0), (0, 0), (1, 1), (1, 1)))
    out[:, :, MARG:MARG + GRID] = grid.reshape(n, IC, GRID).astype(ml_dtypes.bfloat16)
    return np.ascontiguousarray(out)


def _in_maps(x, kernels, a, b):
    wt, a2, ftT, iota8, bias_u = _host_prep(kernels, a, b)
    in_maps = []
    for c in range(NCORES):
        in_maps.append({
            "x": _pad_shard(x[IMG_PER_CORE * c:IMG_PER_CORE * (c + 1)]),
            "wt": wt,
            "a2": a2,
            "ftT": ftT,
            "iota8": iota8,
        })
    return in_maps, bias_u


def _unshard(res):
    """Gather per-core padded bf16 outputs -> full fp32 [16, OC, 56, 56]."""
    out_pad = np.concatenate(
        [np.asarray(res.results[c]["out"], dtype=np.float32) for c in range(NCORES)],
        axis=0)
    return np.ascontiguousarray(
        out_pad.reshape(B_FULL, OC, HP, WP)[:, :, 1:1 + H, 1:1 + W])


def kernel(x, kernels, a, b, mode=0, **_ignored):
    from concourse.bass_utils import run_bass_kernel_spmd

    x = np.ascontiguousarray(np.asarray(x, np.float32))
    kernels = np.asarray(kernels, np.float32)
    a = np.asarray(a, np.float32)

    in_maps, bias_u = _in_maps(x, kernels, a, b)
    nc = _build_graph(bias_u)
    res = run_bass_kernel_spmd(nc, in_maps, core_ids=list(range(NCORES)))
    return _unshard(res)


# revision 22
# speedup vs baseline: 1.0628x; 1.0566x over previous
"""ALSH Conv2d distributed Trainium2 kernel (8 NeuronCores).

Strategy (v4):
  - Data-parallel over batch: 16 images -> 2 per core, one image per
    64-partition half of SBUF (image0 on partitions 0-63, image1 on 64-127).
  - Conv as 9 shifted K=64 matmuls per output chunk, with the two images'
    matmuls issued as concurrent row-tiled pairs (tile_position (0,0) and
    (64,0)) so the 128x128 PE array stays fully busy. bf16 matmul dtype
    (fp32 accumulation in PSUM).
  - Vote-first: t-matmuls (t[r,m] = sum_c a2[c,r] xp[c,m]) run immediately
    after the input loads, one PSUM bank per chunk with image0 rows on
    partitions 0:9 and image1 on 64:73 (deep 6-buf pool in a nested scope
    released before the conv pool allocates, so neither phase stalls on
    PSUM).  Shifted gather-DMAs + DVE adds + magic-number bucketize give
    the per-core 8-bin histogram by ~25us.
  - The global vote skips the CC stream entirely: 7 XOR-relative
    remote_dma_broadcast descriptors (slot j sends to tpb ^ j) push each
    core's [128,8] partial histogram into peer SBUF rx slots; a monotonic
    remote semaphore counts the 14 lane-arrivals.  XOR is a bijection so
    the slot order is a permutation -- summing the slots plus the local
    partials and a gpsimd partition_all_reduce yields the global histogram
    on every partition with no entry barrier and ~2us of latency.
  - argmax -> one-hot -> factor vector against a host-precomputed 128x8
    factor table (the kernel-side hash depends only on the weights, so it
    is computed bit-exactly on host, jax/cpu).
  - Conv outputs are evicted unscaled to bf16 SBUF staging; as soon as the
    vote lands (~30us, under the conv) 3 row groups x 2 images are scaled
    (DVE/ACT alternating) and stored bf16 over the sync/scalar HWDGE
    queues + gpsimd SWDGE, overlapping the conv tail.  The host converts
    to fp32 and strips the padding during the unshard.
"""

import numpy as np

OC, IC, KS, R_LSH, T_TBL = 128, 64, 3, 2.5, 8
SPAN = KS * KS * IC          # 576
B_FULL, H, W = 16, 56, 56
NCORES = 8
IMG_PER_CORE = B_FULL // NCORES   # 2
HP, WP = H + 2, W + 2        # 58x58 padded grid
GRID = HP * WP               # 3364
MARG = 64                    # zero margin columns on each side of the grid
LCOLS = MARG + GRID + MARG   # 3492
# chunk row counts over the 58 padded rows; all chunks >= 256 cols so fp32r
# runs at full rate (464,464,464,464,464,464,290,290)
CHUNK_ROWS = [8, 8, 8, 8, 8, 8, 5, 5]
ROW_START = [0, 8, 16, 24, 32, 40, 48, 53]
NCHUNK = len(CHUNK_ROWS)
MAXCHUNK = 8 * WP            # 464
SHIFTS = [((r // 3) - 1) * WP + ((r % 3) - 1) for r in range(9)]
# scale/store row groups (in padded rows): many small stores so several
# SDMA engines stream concurrently (one engine per in-flight DMA)
GROUPS = [(0, 7), (7, 14), (14, 22), (22, 29), (29, 36), (36, 44), (44, 51), (51, 58)]
USE_RDMA = True              # False: CC-stream AllGather fallback


def _build_graph(bias_u):
    """Build the 8-core Bass graph. bias_u = (0.5*sum(a[576:]) + b) / R."""
    import concourse.bass as bass
    import concourse.bacc as bacc
    import bass_rust
    import concourse.mybir as mybir
    from concourse import bass_isa
    from concourse import tile

    f32 = mybir.dt.float32
    bf16 = mybir.dt.bfloat16
    AX = mybir.AxisListType.X
    AF = mybir.ActivationFunctionType
    OP = mybir.AluOpType

    nc = bacc.Bacc("TRN2", target_bir_lowering=False, debug=False,
                   num_devices=NCORES, num_swdge_queues=2,
                   monotonic_sem_count=2)

    x_d = nc.dram_tensor("x", [IMG_PER_CORE, IC, LCOLS], bf16, kind="ExternalInput")
    wt_d = nc.dram_tensor("wt", [128, 9 * OC], bf16, kind="ExternalInput")
    a2_d = nc.dram_tensor("a2", [128, 9], bf16, kind="ExternalInput")
    ft_d = nc.dram_tensor("ftT", [OC, T_TBL], f32, kind="ExternalInput")
    io_d = nc.dram_tensor("iota8", [128, T_TBL], f32, kind="ExternalInput")
    out_d = nc.dram_tensor("out", [IMG_PER_CORE, OC, GRID], bf16, kind="ExternalOutput")

    with tile.TileContext(nc) as tc:
        with (
            tc.tile_pool(name="const", bufs=1) as cpool,
            tc.tile_pool(name="xp", bufs=1) as xpool,
            tc.tile_pool(name="tg", bufs=1) as tgpool,
            tc.tile_pool(name="stg", bufs=1) as stgpool,
            tc.tile_pool(name="vv", bufs=9) as vvpool,
            tc.tile_pool(name="vsmall", bufs=1) as vspool,
            tc.tile_pool(name="tiny", bufs=1) as typool,
            tc.tile_pool(name="pswu", bufs=1, space="PSUM") as pswu,
            tc.tile_pool(name="dram", bufs=2, space="DRAM") as dram,
        ):
            # ---- constants ----
            w_sb = cpool.tile([128, 9 * OC], bf16, tag="wsb")
            a2_sb = cpool.tile([128, 9], bf16, tag="a2sb")
            ft_sb = cpool.tile([OC, T_TBL], f32, tag="ftsb")
            iota_sb = cpool.tile([128, T_TBL], f32, tag="iosb")

            # ---- padded input xp: [128, LCOLS]; partitions 0-63 image0,
            #      64-127 image1; grid position m lives at column MARG+m.
            # x arrives host-padded to the full [64, LCOLS] row layout
            # (zero margins + zero-padded 58x58 grid). Split over the three
            # DMA queues so the vote's t-matmul can start ASAP. ----
            xp = xpool.tile([128, LCOLS], bf16, tag="xp")
            HALF = LCOLS // 2
            nc.gpsimd.dma_start(a2_sb[:], a2_d[:])
            nc.sync.dma_start(xp[0:64, 0:HALF], x_d[0][:, 0:HALF])
            nc.scalar.dma_start(xp[0:64, HALF:], x_d[0][:, HALF:])
            nc.gpsimd.dma_start(xp[64:128, 0:HALF], x_d[1][:, 0:HALF])
            nc.sync.dma_start(xp[64:128, HALF:], x_d[1][:, HALF:])
            nc.scalar.dma_start(w_sb[:], wt_d[:])
            nc.sync.dma_start(ft_sb[:], ft_d[:])
            nc.sync.dma_start(iota_sb[:], io_d[:])

            # per-core partial histogram [128, 8]; DVE writes rows 0:112,
            # the pad rows are zeroed once up front
            hist = vspool.tile([128, T_TBL], f32, tag="hist")
            nc.gpsimd.memset(hist[:, :], 0.0)
            ones_sb = cpool.tile([128, 128], f32, tag="ones")
            nc.vector.memset(ones_sb[:, :], 1.0)

            # ================= vote stage 1: t[r, m] = sum_c a2[c,r]*xp[c,m] ==
            # one PSUM bank per chunk: image0 rows on partitions 0:9,
            # image1 on 64:73; deep pool in a nested scope (released before
            # the conv pool allocates) so the evict round-trips never stall
            # the PE.
            t_sb = tgpool.tile([128, LCOLS], f32, tag="tsb")
            with tc.tile_pool(name="pst", bufs=6, space="PSUM") as pst:
                for k in range(NCHUNK):
                    ncols = CHUNK_ROWS[k] * WP
                    c0 = MARG + ROW_START[k] * WP
                    tps = pst.tile([128, MAXCHUNK], f32, tag="tps")
                    nc.tensor.matmul(tps[0:9, 0:ncols],
                                     a2_sb[0:64, :],
                                     xp[0:64, c0:c0 + ncols],
                                     start=True, stop=True, tile_position=(0, 0))
                    nc.tensor.matmul(tps[64:73, 0:ncols],
                                     a2_sb[64:128, :],
                                     xp[64:128, c0:c0 + ncols],
                                     start=True, stop=True, tile_position=(64, 64))
                    nc.vector.tensor_copy(t_sb[0:9, c0:c0 + ncols], tps[0:9, 0:ncols])
                    nc.scalar.copy(t_sb[64:73, c0:c0 + ncols], tps[64:73, 0:ncols])
            del tps

            # ---- shifted gathers: vvr[y, x] = t[r, (y+1+dy)*58 + (x+1+dx)] ----
            # split across the sync/scalar HWDGE queues + gpsimd SWDGE
            vvr_tiles = []
            for r in range(9):
                vvr = vvpool.tile([112, W], f32, tag="vvr")
                off = MARG + SHIFTS[r] + WP + 1
                for i in range(IMG_PER_CORE):
                    src = t_sb[64 * i + r:64 * i + r + 1, off:off + H * WP] \
                        .rearrange("p (y x) -> p y x", x=WP)[:, :, 0:W]
                    eng = (nc.sync, nc.gpsimd, nc.scalar)[(2 * r + i) % 3]
                    eng.dma_start(vvr[56 * i:56 * i + 56, :], src)
                vvr_tiles.append(vvr)

            if USE_RDMA:
                # ---- global histogram via XOR-relative remote DMA ----
                # descgens are emitted here, right after the gathers, so the
                # gpsimd absorbs the SWDGE ucode library load while the DVE
                # crunches the vote math; the source read is deferred to the
                # trigger (emitted after the histogram below).  instruction j
                # broadcasts this core's [128,8] partials to tpb^j's rx slot
                # j-1; 7 instructions x 2 lanes bump the monotonic remote sem
                # to 14 once every peer's data landed.
                rx = vspool.tile([128, (NCORES - 1) * T_TBL], f32, tag="rx")
                mono_rx = nc.monotonic_semaphore(0)
                mono_loc = nc.monotonic_semaphore(1)
                for j in range(1, NCORES):
                    rdests = [(0, k) if k == j else None for k in range(NCORES)]
                    nc.gpsimd.remote_dma_broadcast(
                        rx[:, (j - 1) * T_TBL:j * T_TBL],
                        hist[:, :],
                        mono_rx.sem(),
                        mono_loc.sem(),
                        rdests=rdests,
                        queue_num=1,
                    )

            # ---- v = sum_r vvr ; bucketize ; histogram (DVE/ACT, pre-conv) --
            # floor via the magic-number round-to-nearest trick (no floor/mod
            # ALU op): rni(z) = (z + MAGIC) - MAGIC for |z| < 2^22.
            MAGIC = 12582912.0  # 1.5 * 2^23
            acc = vspool.tile([112, W], f32, tag="acc")
            nc.vector.tensor_tensor(acc[:], vvr_tiles[0][:], vvr_tiles[1][:], OP.add)
            for r in range(2, 9):
                nc.vector.tensor_tensor(acc[:], acc[:], vvr_tiles[r][:], OP.add)
            u_t = vspool.tile([112, W], f32, tag="ut")
            nc.vector.tensor_scalar(u_t[:], acc[:], float(1.0 / R_LSH), float(bias_u), OP.mult, OP.add)
            u2 = vspool.tile([112, W], f32, tag="u2")
            nc.vector.tensor_scalar(u2[:], u_t[:], 0.49995, MAGIC, OP.subtract, OP.add)
            q_t = vspool.tile([112, W], f32, tag="qt")
            nc.vector.tensor_scalar(q_t[:], u2[:], MAGIC, None, OP.subtract)
            aq = vspool.tile([112, W], f32, tag="aq")
            nc.vector.scalar_tensor_tensor(aq[:], q_t[:], -1.0, q_t[:], OP.mult, OP.max)
            d1 = vspool.tile([112, W], f32, tag="d1")
            nc.vector.tensor_scalar(d1[:], aq[:], 0.125, 0.499, OP.mult, OP.subtract)
            d2 = vspool.tile([112, W], f32, tag="d2")
            nc.vector.tensor_scalar(d2[:], d1[:], MAGIC, MAGIC, OP.add, OP.subtract)
            votes = vspool.tile([112, W], f32, tag="votes")
            nc.vector.scalar_tensor_tensor(votes[:], d2[:], -8.0, aq[:], OP.mult, OP.add)

            eq = vspool.tile([112, W], f32, tag="eq")
            for t in range(T_TBL):
                nc.vector.tensor_scalar(eq[:], votes[:], float(t), None, OP.is_equal)
                nc.vector.reduce_sum(hist[0:112, t:t + 1], eq[:], AX)

            tot = typool.tile([128, T_TBL], f32, tag="tot")
            if USE_RDMA:
                # fire the pre-built descriptors; the deferred hist read makes
                # this trigger (not the descgens) wait for the histogram
                nc.gpsimd.trigger_dma(count=None, queue_num=1)
            else:
                hist_row = vspool.tile([1, T_TBL], f32, tag="histrow")
                nc.gpsimd.reduce_sum(hist_row[0:1, :], hist[0:112, :], mybir.AxisListType.C)
                cin = dram.tile([1, T_TBL], f32, tag="cin")
                cout = dram.tile([NCORES, T_TBL], f32, tag="cout", addr_space="Shared")
                nc.gpsimd.dma_start(cin[:], hist_row[0:1, :])
                nc.gpsimd.collective_compute(
                    "AllGather",
                    mybir.AluOpType.bypass,
                    replica_groups=[list(range(NCORES))],
                    ins=[cin[:].opt()],
                    outs=[cout[:].opt()],
                )
                hs_bc = typool.tile([128, NCORES * T_TBL], f32, tag="hsbc")
                nc.sync.dma_start(hs_bc[:], cout[:].rearrange("r t -> (r t)")
                                  .unsqueeze(0).broadcast_to([128, NCORES * T_TBL]))

            # ---- PE warm-up between the t-matmuls and the conv so the HAM
            # clock gate is fully open before the conv begins ----
            wups = pswu.tile([128, MAXCHUNK], f32, tag="pswu")
            for wi in range(8):
                nc.tensor.matmul(wups[:, 0:MAXCHUNK],
                                 w_sb[0:128, 0:128],
                                 w_sb[0:128, 0:MAXCHUNK],
                                 start=True, stop=True)

            # ================= main conv =================
            # per-image contiguous bf16 staging over the whole padded grid
            stg_img0 = stgpool.tile([128, GRID], bf16, tag="stg0")
            stg_img1 = stgpool.tile([128, GRID], bf16, tag="stg1")
            stg_imgs = [stg_img0, stg_img1]
            with tc.tile_pool(name="psc", bufs=6, space="PSUM") as psc:
                for k in range(NCHUNK):
                    ncols = CHUNK_ROWS[k] * WP
                    c0 = MARG + ROW_START[k] * WP
                    g0 = ROW_START[k] * WP
                    psA = psc.tile([128, MAXCHUNK], f32, tag="psconv")
                    psB = psc.tile([128, MAXCHUNK], f32, tag="psconv")
                    for r in range(9):
                        s = SHIFTS[r]
                        nc.tensor.matmul(psA[:, 0:ncols],
                                         w_sb[0:64, r * OC:(r + 1) * OC],
                                         xp[0:64, c0 + s:c0 + s + ncols],
                                         start=(r == 0), stop=(r == 8),
                                         tile_position=(0, 0))
                        nc.tensor.matmul(psB[:, 0:ncols],
                                         w_sb[64:128, r * OC:(r + 1) * OC],
                                         xp[64:128, c0 + s:c0 + s + ncols],
                                         start=(r == 0), stop=(r == 8),
                                         tile_position=(64, 0))
                    # both evictions on ACT: the DVE stays clear for the
                    # vote/histogram chain feeding the remote exchange
                    nc.scalar.copy(stg_imgs[0][:, g0:g0 + ncols], psA[:, 0:ncols])
                    nc.scalar.copy(stg_imgs[1][:, g0:g0 + ncols], psB[:, 0:ncols])

            # ---- complete the vote ----
            if USE_RDMA:
                # wait for all 7 peers' data, then sum the permuted slots
                # plus the local partials; the no-sync barrier pins the
                # gpsimd consumers behind the manual semaphore wait (Tile
                # cannot see the remote writes).
                mono_rx.wait_inc(14)
                tc.no_sync_barrier()
                nc.gpsimd.tensor_tensor(tot[:], rx[:, 0:T_TBL],
                                        rx[:, T_TBL:2 * T_TBL], OP.add)
                for j in range(3, NCORES):
                    nc.gpsimd.tensor_tensor(tot[:], tot[:],
                                            rx[:, (j - 1) * T_TBL:j * T_TBL], OP.add)
                nc.gpsimd.tensor_tensor(tot[:], tot[:], hist[:], OP.add)
                # cross-partition total via a ones-matmul on the (idle) PE:
                # partition_all_reduce would force a gpsimd ucode library
                # swap that waits out the async SWDGE descgen completions
                nc.tensor.matmul(wups[:, 0:T_TBL], ones_sb[:, :], tot[:, :],
                                 start=True, stop=True)
                tot_ap = wups[:, 0:T_TBL]
            else:
                hs_v = hs_bc[:, :].rearrange("p (r t) -> p t r", t=T_TBL)
                tot_all = typool.tile([128, T_TBL], f32, tag="totall")
                nc.vector.reduce_sum(tot_all[:], hs_v, AX)
                tot_ap = tot_all[:]

            # ---- argmax -> one-hot -> factor vector ----
            score = typool.tile([128, T_TBL], f32, tag="score")
            nc.vector.scalar_tensor_tensor(score[:], tot_ap, float(T_TBL), iota_sb[:],
                                           OP.mult, OP.subtract)
            mx = typool.tile([128, 1], f32, tag="mx")
            nc.vector.reduce_max(mx[:], score[:], AX)
            eqb = typool.tile([128, T_TBL], f32, tag="eqb")
            nc.vector.tensor_scalar(eqb[:], score[:], mx[:, 0:1], None, OP.is_equal)
            fvt = typool.tile([128, T_TBL], f32, tag="fvt")
            nc.vector.tensor_tensor(fvt[:], ft_sb[:], eqb[:], OP.mult)
            fv_sb = typool.tile([128, 1], f32, tag="fvsb")
            nc.vector.reduce_sum(fv_sb[:], fvt[:], AX)

            # ---- scale by factor vector, then DMA out; 3 row groups x 2
            # images pipelined over DVE/ACT and three DMA queues ----
            ei = 0
            for gi, (r0, r1) in enumerate(GROUPS):
                for i in range(IMG_PER_CORE):
                    stg = stg_imgs[i]
                    if ei % 2 == 0:
                        nc.vector.tensor_scalar(stg[:, r0 * WP:r1 * WP],
                                                stg[:, r0 * WP:r1 * WP],
                                                fv_sb[:, 0:1], None, OP.mult)
                    else:
                        nc.scalar.activation(stg[:, r0 * WP:r1 * WP],
                                             stg[:, r0 * WP:r1 * WP],
                                             AF.Copy, scale=fv_sb[:, 0:1])
                    # contiguous padded-grid store; host strips the padding
                    oeng = (nc.sync, nc.scalar, nc.gpsimd)[ei % 3]
                    oeng.dma_start(out_d[i, :, r0 * WP:r1 * WP],
                                   stg[:, r0 * WP:r1 * WP])
                    ei += 1

    nc.compile()
    return nc


def _host_prep(kernels, a, b):
    """Host-side weight layouts + bit-exact factor table via jax on CPU."""
    import jax
    import jax.numpy as jnp

    cpu = jax.devices("cpu")[0]
    k_j = jax.device_put(jnp.asarray(kernels, jnp.float32), cpu)
    a_j = jax.device_put(jnp.asarray(a, jnp.float32), cpu)
    b_j = jax.device_put(jnp.asarray(b, jnp.float32), cpu)

    norms2 = jnp.sum(k_j * k_j, axis=1)
    powers = jnp.stack([norms2 ** (2 ** i) for i in range(5)], axis=1)
    hk = k_j @ a_j[:SPAN] + powers @ a_j[SPAN:]
    kidx = np.asarray(jnp.abs(jnp.fmod(jnp.floor((hk + b_j) / R_LSH).astype(jnp.int32), T_TBL)))

    ftT = np.zeros((T_TBL, OC), np.float32)
    for t in range(T_TBL):
        mask = (kidx == t).astype(np.float32)
        cnt = mask.sum()
        if cnt > 0:
            ftT[t] = mask * np.float32(OC / max(cnt, np.float32(1.0)))
        else:
            ftT[t] = 1.0
    ftT = np.ascontiguousarray(ftT.T)  # [OC, T_TBL], oc on partitions

    c0 = 0.5 * float(jnp.sum(a_j[SPAN:]))
    bias_u = (c0 + float(b_j)) / R_LSH

    import ml_dtypes
    wt_half = np.asarray(kernels, np.float32).reshape(OC, IC, 9).transpose(1, 2, 0)  # [64, 9, 128]
    wt = np.concatenate([wt_half, wt_half], axis=0).reshape(128, 9 * OC)
    wt = np.ascontiguousarray(wt.astype(ml_dtypes.bfloat16))

    a2_half = np.asarray(a, np.float32)[:SPAN].reshape(IC, 9)
    a2 = np.ascontiguousarray(
        np.concatenate([a2_half, a2_half], axis=0).astype(ml_dtypes.bfloat16))

    iota8 = np.ascontiguousarray(np.tile(np.arange(T_TBL, dtype=np.float32), (128, 1)))
    return wt, a2, ftT, iota8, bias_u


def _pad_shard(xs):
    """[n, 64, 56, 56] -> bf16 [n, 64, LCOLS]: margins + padded 58x58 grid."""
    import ml_dtypes
    n = xs.shape[0]
    out = np.zeros((n, IC, LCOLS), ml_dtypes.bfloat16)
    grid = np.pad(xs, ((0, 0), (0, 0), (1, 1), (1, 1)))
    out[:, :, MARG:MARG + GRID] = grid.reshape(n, IC, GRID).astype(ml_dtypes.bfloat16)
    return np.ascontiguousarray(out)


def _in_maps(x, kernels, a, b):
    wt, a2, ftT, iota8, bias_u = _host_prep(kernels, a, b)
    in_maps = []
    for c in range(NCORES):
        in_maps.append({
            "x": _pad_shard(x[IMG_PER_CORE * c:IMG_PER_CORE * (c + 1)]),
            "wt": wt,
            "a2": a2,
            "ftT": ftT,
            "iota8": iota8,
        })
    return in_maps, bias_u


def _unshard(res):
    """Gather per-core padded bf16 outputs -> full fp32 [16, OC, 56, 56]."""
    out_pad = np.concatenate(
        [np.asarray(res.results[c]["out"], dtype=np.float32) for c in range(NCORES)],
        axis=0)
    return np.ascontiguousarray(
        out_pad.reshape(B_FULL, OC, HP, WP)[:, :, 1:1 + H, 1:1 + W])


def kernel(x, kernels, a, b, mode=0, **_ignored):
    from concourse.bass_utils import run_bass_kernel_spmd

    x = np.ascontiguousarray(np.asarray(x, np.float32))
    kernels = np.asarray(kernels, np.float32)
    a = np.asarray(a, np.float32)

    in_maps, bias_u = _in_maps(x, kernels, a, b)
    nc = _build_graph(bias_u)
    res = run_bass_kernel_spmd(nc, in_maps, core_ids=list(range(NCORES)))
    return _unshard(res)
